# revision 2
# baseline (speedup 1.0000x reference)
"""ConditionalSelfAttention (B=8, C=256, H=W=64, QK=32, LC=32) on 8 TRN2 NeuronCores.

Data-parallel over batch: core b computes batch element b.

Per-core program:
  xf (256, 4096) fp32 (+ bf16 copy for matmul inputs)
  q = Wq@xf + bq, k = Wk@xf + bk            (bf16, replicated 4x across partition
                                             groups for row-tiled K=32 matmuls)
  vT[j, c] = sum_c' xf[c', j] Wv.T[c', c] + bv[c]   (bias via ones-row matmul)
  energyT[j, i] = sum_d k[d, j] q[d, i]     (4 concurrent tile_position matmuls)
  PT = exp(energyT - SHIFT)                 (ScalarE, PSUM -> SBUF, bf16)
  out_T[i, c] = sum_j PT[j, i] vT_ext[j, c] (vT_ext col 256 == 1 gives the
                                             softmax denominator l_i for free)
  out[c, i] = gamma/l_i * out_T[i, c] + xf[c, i]   (PE transpose + fp32 residual)
  rows 256:288 = broadcast(softmax(label) @ We.T + be)   (fp32)
"""

import numpy as np

import concourse.bass as bass
import concourse.bacc as bacc
import concourse.mybir as mybir
import concourse.tile as tile
from concourse.bass_utils import run_bass_kernel_spmd
from concourse.masks import make_identity

F32 = mybir.dt.float32
BF16 = mybir.dt.bfloat16
AF = mybir.ActivationFunctionType

B, C, HW, N = 8, 256, 64, 4096
QK, LC = 32, 32
COUT = C + LC  # 288
SHIFT = 24.0

IC = 512          # i-chunk for the energy/exp phase
NIC = N // IC     # 8
NJB = N // 128    # 32 j-blocks
PT_BUFS = 12      # PT (128, 2048) bf16 slots -> 48KB/partition


def host_prep(x_b, label_b, Wq, bq, Wk, bk, Wv, bv, gamma, We, be):
    """Per-core input dict. x_b: (C, H, W); label_b: (LC,)."""
    xf = np.ascontiguousarray(x_b.reshape(C, N).astype(np.float32))
    wq4 = np.ascontiguousarray(np.tile(Wq.T, (1, 4)).astype(np.float32))
    wk4 = np.ascontiguousarray(np.tile(Wk.T, (1, 4)).astype(np.float32))
    bq4 = np.ascontiguousarray(np.tile(bq, 4)[:, None].astype(np.float32))
    bk4 = np.ascontiguousarray(np.tile(bk, 4)[:, None].astype(np.float32))
    wv_ext = np.ascontiguousarray(
        np.concatenate([Wv.T, bv[None, :]], axis=0).astype(np.float32))
    we_ext = np.ascontiguousarray(
        np.concatenate([We.T, be[None, :]], axis=0).astype(np.float32))
    return {
        "x": xf,
        "wq4": wq4,
        "wk4": wk4,
        "bq4": bq4,
        "bk4": bk4,
        "wv_ext": wv_ext,
        "we_ext": we_ext,
        "label": np.ascontiguousarray(label_b[None, :].astype(np.float32)),
        "gamma": np.ascontiguousarray(np.asarray(gamma, np.float32).reshape(1, 1)),
    }


def build_program(nc, tc):
    x_d = nc.dram_tensor("x", [C, N], F32, kind="ExternalInput")
    wq4_d = nc.dram_tensor("wq4", [C, 128], F32, kind="ExternalInput")
    wk4_d = nc.dram_tensor("wk4", [C, 128], F32, kind="ExternalInput")
    bq4_d = nc.dram_tensor("bq4", [128, 1], F32, kind="ExternalInput")
    bk4_d = nc.dram_tensor("bk4", [128, 1], F32, kind="ExternalInput")
    wv_d = nc.dram_tensor("wv_ext", [C + 1, C], F32, kind="ExternalInput")
    we_d = nc.dram_tensor("we_ext", [LC + 1, LC], F32, kind="ExternalInput")
    lbl_d = nc.dram_tensor("label", [1, LC], F32, kind="ExternalInput")
    gam_d = nc.dram_tensor("gamma", [1, 1], F32, kind="ExternalInput")
    out_d = nc.dram_tensor("out", [COUT, N], F32, kind="ExternalOutput")

    from contextlib import ExitStack
    ctx = ExitStack()
    cpool = ctx.enter_context(tc.tile_pool(name="consts", bufs=1))
    work = ctx.enter_context(tc.tile_pool(name="work", bufs=1))
    pspool = ctx.enter_context(tc.tile_pool(name="ps", bufs=1, space="PSUM"))

    # ---- constants / weights ----
    ident = cpool.tile([128, 128], F32, name="ident")
    make_identity(nc, ident)
    ones_row = cpool.tile([1, 128], BF16, name="ones_row")
    nc.gpsimd.memset(ones_row[:], 1.0)
    one1 = cpool.tile([1, 1], F32, name="one1")
    nc.gpsimd.memset(one1[:], 1.0)
    nshift = cpool.tile([128, 1], F32, name="nshift")
    nc.gpsimd.memset(nshift[:], -SHIFT)

    # fp32 staging for weights, then bf16 casts for matmul inputs
    wq4f = cpool.tile([128, 2 * 128], F32, name="wq4f")
    nc.sync.dma_start(wq4f[:].rearrange("p (kc m) -> p kc m", kc=2),
                      wq4_d.rearrange("(kc p) m -> p kc m", kc=2))
    wk4f = cpool.tile([128, 2 * 128], F32, name="wk4f")
    nc.sync.dma_start(wk4f[:].rearrange("p (kc m) -> p kc m", kc=2),
                      wk4_d.rearrange("(kc p) m -> p kc m", kc=2))
    wvf = cpool.tile([128, 2 * C], F32, name="wvf")
    nc.sync.dma_start(wvf[:].rearrange("p (kc m) -> p kc m", kc=2),
                      wv_d[0:256, :].rearrange("(kc p) m -> p kc m", kc=2))
    bv_rowf = cpool.tile([1, C], F32, name="bv_rowf")
    nc.sync.dma_start(bv_rowf[:], wv_d[256:257, :])
    wq4 = cpool.tile([128, 2 * 128], BF16, name="wq4")
    nc.vector.tensor_copy(wq4[:], wq4f[:])
    wk4 = cpool.tile([128, 2 * 128], BF16, name="wk4")
    nc.vector.tensor_copy(wk4[:], wk4f[:])
    wv = cpool.tile([128, 2 * C], BF16, name="wv")
    nc.vector.tensor_copy(wv[:], wvf[:])
    bv_row = cpool.tile([1, C], BF16, name="bv_row")
    nc.vector.tensor_copy(bv_row[:], bv_rowf[:])

    bq4 = cpool.tile([128, 1], F32, name="bq4")
    nc.sync.dma_start(bq4[:], bq4_d[:])
    bk4 = cpool.tile([128, 1], F32, name="bk4")
    nc.sync.dma_start(bk4[:], bk4_d[:])
    we = cpool.tile([LC + 1, LC], F32, name="we")
    nc.sync.dma_start(we[:], we_d[:])
    gam = cpool.tile([128, 1], F32, name="gam")
    nc.sync.dma_start(gam[:], gam_d[:].to_broadcast((128, 1)))

    # ---- xf: fp32 (residual) + bf16 copy (matmul inputs), chunked for overlap ----
    xf = []
    xfb = []
    for cc in range(2):
        t = work.tile([128, N], F32, name=f"xf{cc}")
        tb = work.tile([128, N], BF16, name=f"xfb{cc}")
        for ch in range(4):
            sl = bass.ts(ch, N // 4)
            nc.sync.dma_start(t[:, sl], x_d[cc * 128:(cc + 1) * 128, sl])
            nc.vector.tensor_copy(tb[:, sl], t[:, sl])
        xf.append(t)
        xfb.append(tb)

    # ---- label branch (fp32) ----
    lbl = work.tile([1, LC], F32, name="lbl")
    nc.sync.dma_start(lbl[:], lbl_d[:])
    lmax = work.tile([1, 1], F32, name="lmax")
    nc.vector.reduce_max(lmax[:], lbl[:], axis=mybir.AxisListType.X)
    nlmax = work.tile([1, 1], F32, name="nlmax")
    nc.vector.tensor_scalar_mul(nlmax[:], lmax[:], -1.0)
    lexp = work.tile([1, LC], F32, name="lexp")
    nc.scalar.activation(lexp[:], lbl[:], AF.Exp, bias=nlmax[:], scale=1.0)
    lsum = work.tile([1, 1], F32, name="lsum")
    nc.vector.reduce_sum(lsum[:], lexp[:], axis=mybir.AxisListType.X)
    lrec = work.tile([1, 1], F32, name="lrec")
    nc.vector.reciprocal(lrec[:], lsum[:])
    srow = work.tile([1, LC], F32, name="srow")
    nc.vector.tensor_scalar_mul(srow[:], lexp[:], lrec[:])
    scol_ps = pspool.tile([LC, 1], F32, name="scol_ps", tag="ops", bufs=3)
    nc.tensor.matmul(scol_ps[:], srow[:], one1[:], start=True, stop=True)
    sext = work.tile([LC + 1, 1], F32, name="sext")
    nc.vector.tensor_copy(sext[0:LC, :], scol_ps[:])
    nc.gpsimd.memset(sext[LC:LC + 1, :], 1.0)
    u_ps = pspool.tile([LC, 1], F32, name="u_ps", tag="ops", bufs=3)
    nc.tensor.matmul(u_ps[:], we[:], sext[:], start=True, stop=True)
    u_sb = work.tile([LC, 1], F32, name="u_sb")
    nc.vector.tensor_copy(u_sb[:], u_ps[:])
    for t in range(NIC):
        u_bc = work.tile([LC, IC], F32, name="u_bc", tag="u_bc", bufs=2)
        nc.scalar.activation(u_bc[:], xf[0][0:LC, bass.ts(t, IC)],
                             AF.Identity, bias=u_sb[:], scale=0.0)
        nc.sync.dma_start(out_d[C:COUT, bass.ts(t, IC)], u_bc[:])

    # ---- projections q4, k4 (bf16 out) ----
    q4 = work.tile([128, N], BF16, name="q4")
    k4 = work.tile([128, N], BF16, name="k4")
    for t in range(NIC):
        sl = bass.ts(t, IC)
        qk_ps = pspool.tile([128, 4 * IC], F32, name="qk_ps", tag="eps", bufs=1)
        q_ps = qk_ps[:, 0:IC]
        k_ps = qk_ps[:, IC:2 * IC]
        for kc in range(2):
            nc.tensor.matmul(
                q_ps, wq4[:, bass.ts(kc, 128)], xfb[kc][:, sl],
                start=(kc == 0), stop=(kc == 1))
        for kc in range(2):
            nc.tensor.matmul(
                k_ps, wk4[:, bass.ts(kc, 128)], xfb[kc][:, sl],
                start=(kc == 0), stop=(kc == 1))
        nc.vector.tensor_scalar_add(q4[:, sl], q_ps, bq4[:])
        nc.vector.tensor_scalar_add(k4[:, sl], k_ps, bk4[:])

    # ---- vT_ext (bf16) ----
    vt = []
    v_ps_big = None
    for jb in range(NJB):
        if jb % 4 == 0:
            v_ps_big = pspool.tile([128, 4 * IC], F32, name="v_ps_big",
                                   tag="eps", bufs=1)
        v_ps = v_ps_big[:, (jb % 4) * IC:(jb % 4) * IC + C]
        for kc in range(2):
            nc.tensor.matmul(
                v_ps, xfb[kc][:, bass.ts(jb, 128)], wv[:, bass.ts(kc, C)],
                start=(kc == 0), stop=False)
        nc.tensor.matmul(v_ps, ones_row[:], bv_row[:], start=False, stop=True)
        t = work.tile([128, C + 1], BF16, name=f"vt{jb}", tag="vt", bufs=NJB)
        nc.vector.tensor_copy(t[:, 0:C], v_ps)
        nc.gpsimd.memset(t[:, C:C + 1], 1.0)
        vt.append(t)

    # ---- attention main loop ----
    for ic in range(NIC):
        isl = bass.ts(ic, IC)
        pt_tiles = []
        for r in range(8):
            e_ps = pspool.tile([128, 4 * IC], F32, name="e_ps", tag="eps", bufs=1)
            for g in range(4):
                jb = g * 8 + r
                nc.tensor.matmul(
                    e_ps[:, bass.ts(g, IC)],
                    k4[32 * g:32 * (g + 1), bass.ts(jb, 128)],
                    q4[32 * g:32 * (g + 1), isl],
                    start=True, stop=True, tile_position=(32 * g, 0))
            pt = work.tile([128, 4 * IC], BF16, name="pt", tag="pt", bufs=PT_BUFS)
            nc.scalar.activation(pt[:], e_ps[:], AF.Exp, bias=nshift[:], scale=1.0)
            pt_tiles.append(pt)

        obs = [work.tile([128, IC], F32, name=f"ob{cc}_{ic}", tag=f"ob{cc}", bufs=2)
               for cc in range(2)]
        for isub in range(4):
            o_ps = pspool.tile([128, C + 1], F32, name="o_ps", tag="ops", bufs=3)
            for jb in range(NJB):
                g, r = jb // 8, jb % 8
                lhsT = pt_tiles[r][:, g * IC + isub * 128: g * IC + (isub + 1) * 128]
                nc.tensor.matmul(o_ps[:], lhsT, vt[jb][:],
                                 start=(jb == 0), stop=(jb == NJB - 1))
            rec = work.tile([128, 1], F32, name="rec", tag="rec", bufs=4)
            nc.vector.reciprocal(rec[:], o_ps[:, C:C + 1])
            osc = work.tile([128, C], F32, name="osc", tag="osc", bufs=3)
            nc.vector.tensor_scalar(
                osc[:], o_ps[:, 0:C], rec[:], gam[:],
                op0=mybir.AluOpType.mult, op1=mybir.AluOpType.mult)
            for cc in range(2):
                tp_ps = pspool.tile([128, 128], F32, name="tp_ps", tag="ops", bufs=3)
                nc.tensor.transpose(tp_ps[:], osc[:, bass.ts(cc, 128)], ident[:])
                nc.vector.tensor_add(
                    obs[cc][:, bass.ts(isub, 128)], tp_ps[:],
                    xf[cc][:, ic * IC + isub * 128: ic * IC + (isub + 1) * 128])
        for cc in range(2):
            nc.sync.dma_start(out_d[cc * 128:(cc + 1) * 128, isl], obs[cc][:])

    ctx.close()


_COMPILED = None


def _get_compiled():
    global _COMPILED
    if _COMPILED is None:
        nc = bacc.Bacc("TRN2", target_bir_lowering=False, debug=False)
        with tile.TileContext(nc) as tc:
            build_program(nc, tc)
        nc.compile()
        _COMPILED = nc
    return _COMPILED


def kernel(x, label, Wq, bq, Wk, bk, Wv, bv, gamma, We, be, _trace=False):
    x = np.asarray(x, np.float32)
    label = np.asarray(label, np.float32)
    Wq, bq = np.asarray(Wq, np.float32), np.asarray(bq, np.float32)
    Wk, bk = np.asarray(Wk, np.float32), np.asarray(bk, np.float32)
    Wv, bv = np.asarray(Wv, np.float32), np.asarray(bv, np.float32)
    gamma = np.asarray(gamma, np.float32)
    We, be = np.asarray(We, np.float32), np.asarray(be, np.float32)

    nc = _get_compiled()
    in_maps = [host_prep(x[b], label[b], Wq, bq, Wk, bk, Wv, bv, gamma, We, be)
               for b in range(B)]
    res = run_bass_kernel_spmd(nc, in_maps, list(range(B)), trace=_trace)
    out = np.stack([res.results[b]["out"] for b in range(B)])
    out = out.reshape(B, COUT, HW, HW).astype(np.float32)
    if _trace:
        return out, res
    return out


# revision 4
# speedup vs baseline: 1.4783x; 1.4783x over previous
"""ConditionalSelfAttention (B=8, C=256, H=W=64, QK=32, LC=32) on 8 TRN2 NeuronCores.

Data-parallel over batch: core b computes batch element b.

Per-core program:
  xf (256, 4096) fp32 (+ bf16 copy for matmul inputs)
  q = Wq@xf + bq, k = Wk@xf + bk            (bf16, replicated 4x across partition
                                             groups for row-tiled K=32 matmuls)
  vT[j, c] = sum_c' xf[c', j] Wv.T[c', c] + bv[c]   (bias via ones-row matmul)
  energyT[j, i] = sum_d k[d, j] q[d, i]     (4 concurrent tile_position matmuls)
  PT = exp(energyT - SHIFT)                 (ScalarE, PSUM -> SBUF, bf16)
  out_T[i, c] = sum_j PT[j, i] vT_ext[j, c] (vT_ext col 256 == 1 gives the
                                             softmax denominator l_i for free)
  out[c, i] = gamma/l_i * out_T[i, c] + xf[c, i]   (PE transpose + fp32 residual)
  rows 256:288 = broadcast(softmax(label) @ We.T + be)   (fp32)
"""

import numpy as np

import concourse.bass as bass
import concourse.bacc as bacc
import concourse.mybir as mybir
import concourse.tile as tile
from concourse.bass_utils import run_bass_kernel_spmd
from concourse.masks import make_identity

F32 = mybir.dt.float32
BF16 = mybir.dt.bfloat16
AF = mybir.ActivationFunctionType

B, C, HW, N = 8, 256, 64, 4096
QK, LC = 32, 32
COUT = C + LC  # 288
SHIFT = 24.0

IC = 512          # i-chunk for the energy/exp phase
NIC = N // IC     # 8
NJB = N // 128    # 32 j-blocks
PT_BUFS = 24      # PT (128, 1024) bf16 slots -> 48KB/partition


def host_prep(x_b, label_b, Wq, bq, Wk, bk, Wv, bv, gamma, We, be):
    """Per-core input dict. x_b: (C, H, W); label_b: (LC,)."""
    xf = np.ascontiguousarray(x_b.reshape(C, N).astype(np.float32))
    wq4 = np.ascontiguousarray(np.tile(Wq.T, (1, 4)).astype(np.float32))
    wk4 = np.ascontiguousarray(np.tile(Wk.T, (1, 4)).astype(np.float32))
    bq4 = np.ascontiguousarray(np.tile(bq, 4)[:, None].astype(np.float32))
    bk4 = np.ascontiguousarray(np.tile(bk, 4)[:, None].astype(np.float32))
    wv_ext = np.ascontiguousarray(
        np.concatenate([Wv.T, bv[None, :]], axis=0).astype(np.float32))
    we_ext = np.ascontiguousarray(
        np.concatenate([We.T, be[None, :]], axis=0).astype(np.float32))
    return {
        "x": xf,
        "wq4": wq4,
        "wk4": wk4,
        "bq4": bq4,
        "bk4": bk4,
        "wv_ext": wv_ext,
        "we_ext": we_ext,
        "label": np.ascontiguousarray(label_b[None, :].astype(np.float32)),
        "gamma": np.ascontiguousarray(np.asarray(gamma, np.float32).reshape(1, 1)),
    }


def build_program(nc, tc):
    x_d = nc.dram_tensor("x", [C, N], F32, kind="ExternalInput")
    wq4_d = nc.dram_tensor("wq4", [C, 128], F32, kind="ExternalInput")
    wk4_d = nc.dram_tensor("wk4", [C, 128], F32, kind="ExternalInput")
    bq4_d = nc.dram_tensor("bq4", [128, 1], F32, kind="ExternalInput")
    bk4_d = nc.dram_tensor("bk4", [128, 1], F32, kind="ExternalInput")
    wv_d = nc.dram_tensor("wv_ext", [C + 1, C], F32, kind="ExternalInput")
    we_d = nc.dram_tensor("we_ext", [LC + 1, LC], F32, kind="ExternalInput")
    lbl_d = nc.dram_tensor("label", [1, LC], F32, kind="ExternalInput")
    gam_d = nc.dram_tensor("gamma", [1, 1], F32, kind="ExternalInput")
    out_d = nc.dram_tensor("out", [COUT, N], F32, kind="ExternalOutput")

    from contextlib import ExitStack
    ctx = ExitStack()
    cpool = ctx.enter_context(tc.tile_pool(name="consts", bufs=1))
    work = ctx.enter_context(tc.tile_pool(name="work", bufs=1))
    pspool = ctx.enter_context(tc.tile_pool(name="ps", bufs=1, space="PSUM"))

    # ---- constants / weights ----
    ident = cpool.tile([128, 128], F32, name="ident")
    make_identity(nc, ident)
    ones_row = cpool.tile([1, 128], BF16, name="ones_row")
    nc.gpsimd.memset(ones_row[:], 1.0)
    one1 = cpool.tile([1, 1], F32, name="one1")
    nc.gpsimd.memset(one1[:], 1.0)
    nshift = cpool.tile([128, 1], F32, name="nshift")
    nc.gpsimd.memset(nshift[:], -SHIFT)

    # fp32 staging for weights, then bf16 casts for matmul inputs
    wq4f = cpool.tile([128, 2 * 128], F32, name="wq4f")
    nc.sync.dma_start(wq4f[:].rearrange("p (kc m) -> p kc m", kc=2),
                      wq4_d.rearrange("(kc p) m -> p kc m", kc=2))
    wk4f = cpool.tile([128, 2 * 128], F32, name="wk4f")
    nc.sync.dma_start(wk4f[:].rearrange("p (kc m) -> p kc m", kc=2),
                      wk4_d.rearrange("(kc p) m -> p kc m", kc=2))
    wvf = cpool.tile([128, 2 * C], F32, name="wvf")
    nc.sync.dma_start(wvf[:].rearrange("p (kc m) -> p kc m", kc=2),
                      wv_d[0:256, :].rearrange("(kc p) m -> p kc m", kc=2))
    bv_rowf = cpool.tile([1, C], F32, name="bv_rowf")
    nc.sync.dma_start(bv_rowf[:], wv_d[256:257, :])
    wq4 = cpool.tile([128, 2 * 128], BF16, name="wq4")
    nc.vector.tensor_copy(wq4[:], wq4f[:])
    wk4 = cpool.tile([128, 2 * 128], BF16, name="wk4")
    nc.vector.tensor_copy(wk4[:], wk4f[:])
    wv = cpool.tile([128, 2 * C], BF16, name="wv")
    nc.vector.tensor_copy(wv[:], wvf[:])
    bv_row = cpool.tile([1, C], BF16, name="bv_row")
    nc.vector.tensor_copy(bv_row[:], bv_rowf[:])

    bq4 = cpool.tile([128, 1], F32, name="bq4")
    nc.sync.dma_start(bq4[:], bq4_d[:])
    bk4 = cpool.tile([128, 1], F32, name="bk4")
    nc.sync.dma_start(bk4[:], bk4_d[:])
    we = cpool.tile([LC + 1, LC], F32, name="we")
    nc.sync.dma_start(we[:], we_d[:])
    gam = cpool.tile([128, 1], F32, name="gam")
    nc.sync.dma_start(gam[:], gam_d[:].to_broadcast((128, 1)))

    # ---- xf: fp32 (residual) + bf16 copy (matmul inputs), chunked for overlap ----
    xf = []
    xfb = []
    for cc in range(2):
        t = work.tile([128, N], F32, name=f"xf{cc}")
        tb = work.tile([128, N], BF16, name=f"xfb{cc}")
        for ch in range(4):
            sl = bass.ts(ch, N // 4)
            nc.sync.dma_start(t[:, sl], x_d[cc * 128:(cc + 1) * 128, sl])
            nc.vector.tensor_copy(tb[:, sl], t[:, sl])
        xf.append(t)
        xfb.append(tb)

    # ---- label branch (fp32) ----
    lbl = work.tile([1, LC], F32, name="lbl")
    nc.sync.dma_start(lbl[:], lbl_d[:])
    lmax = work.tile([1, 1], F32, name="lmax")
    nc.vector.reduce_max(lmax[:], lbl[:], axis=mybir.AxisListType.X)
    nlmax = work.tile([1, 1], F32, name="nlmax")
    nc.vector.tensor_scalar_mul(nlmax[:], lmax[:], -1.0)
    lexp = work.tile([1, LC], F32, name="lexp")
    nc.scalar.activation(lexp[:], lbl[:], AF.Exp, bias=nlmax[:], scale=1.0)
    lsum = work.tile([1, 1], F32, name="lsum")
    nc.vector.reduce_sum(lsum[:], lexp[:], axis=mybir.AxisListType.X)
    lrec = work.tile([1, 1], F32, name="lrec")
    nc.vector.reciprocal(lrec[:], lsum[:])
    srow = work.tile([1, LC], F32, name="srow")
    nc.vector.tensor_scalar_mul(srow[:], lexp[:], lrec[:])
    scol_ps = pspool.tile([LC, 1], F32, name="scol_ps", tag="ops", bufs=2)
    nc.tensor.matmul(scol_ps[:], srow[:], one1[:], start=True, stop=True)
    sext = work.tile([LC + 1, 1], F32, name="sext")
    nc.vector.tensor_copy(sext[0:LC, :], scol_ps[:])
    nc.gpsimd.memset(sext[LC:LC + 1, :], 1.0)
    u_ps = pspool.tile([LC, 1], F32, name="u_ps", tag="ops", bufs=2)
    nc.tensor.matmul(u_ps[:], we[:], sext[:], start=True, stop=True)
    u_sb = work.tile([LC, 1], F32, name="u_sb")
    nc.vector.tensor_copy(u_sb[:], u_ps[:])

    # ---- projections q4, k4 (bf16 out) ----
    q4 = work.tile([128, N], BF16, name="q4")
    k4 = work.tile([128, N], BF16, name="k4")
    for t in range(NIC):
        sl = bass.ts(t, IC)
        qk_ps = pspool.tile([128, 2 * IC], F32, name="qk_ps", tag="eps", bufs=3)
        q_ps = qk_ps[:, 0:IC]
        k_ps = qk_ps[:, IC:2 * IC]
        for kc in range(2):
            nc.tensor.matmul(
                q_ps, wq4[:, bass.ts(kc, 128)], xfb[kc][:, sl],
                start=(kc == 0), stop=(kc == 1))
        for kc in range(2):
            nc.tensor.matmul(
                k_ps, wk4[:, bass.ts(kc, 128)], xfb[kc][:, sl],
                start=(kc == 0), stop=(kc == 1))
        nc.vector.tensor_scalar_add(q4[:, sl], q_ps, bq4[:])
        nc.vector.tensor_scalar_add(k4[:, sl], k_ps, bk4[:])

    # ---- helpers: energy round emission (2 psum halves, 4 row-tiled matmuls) ----
    def emit_energy_round(ic, r, pt_list):
        for half in range(2):
            e_ps = pspool.tile([128, 2 * IC], F32, name="e_ps", tag="eps", bufs=3)
            for gh in range(2):
                g = half * 2 + gh
                jb = g * 8 + r
                nc.tensor.matmul(
                    e_ps[:, bass.ts(gh, IC)],
                    k4[32 * g:32 * (g + 1), bass.ts(jb, 128)],
                    q4[32 * g:32 * (g + 1), bass.ts(ic, IC)],
                    start=True, stop=True, tile_position=(32 * g, 0))
            pt = work.tile([128, 2 * IC], BF16, name="pt", tag="pt", bufs=PT_BUFS)
            nc.scalar.activation(pt[:], e_ps[:], AF.Exp, bias=nshift[:], scale=1.0)
            pt_list.append(pt)

    def pt_slice(pt_list, jb, isub):
        g, r = jb // 8, jb % 8
        t = pt_list[2 * r + g // 2]
        base = (g % 2) * IC + isub * 128
        return t[:, base:base + 128]

    # ---- vT_ext (bf16), interleaved with energy rounds for ic=0 ----
    vt = []
    pt_cur = []
    v_ps_big = None
    for jb in range(NJB):
        if jb % 2 == 0:
            v_ps_big = pspool.tile([128, 2 * IC], F32, name="v_ps_big",
                                   tag="eps", bufs=3)
        # two bank-aligned C-wide regions per psum tile (offsets 0 and 2KB)
        v_ps = v_ps_big[:, (jb % 2) * IC:(jb % 2) * IC + C]
        for kc in range(2):
            nc.tensor.matmul(
                v_ps, xfb[kc][:, bass.ts(jb, 128)], wv[:, bass.ts(kc, C)],
                start=(kc == 0), stop=False)
        nc.tensor.matmul(v_ps, ones_row[:], bv_row[:], start=False, stop=True)
        t = work.tile([128, C + 1], BF16, name=f"vt{jb}", tag="vt", bufs=NJB)
        nc.vector.tensor_copy(t[:, 0:C], v_ps)
        nc.gpsimd.memset(t[:, C:C + 1], 1.0)
        vt.append(t)
        if jb % 4 == 3:
            emit_energy_round(0, jb // 4, pt_cur)

    # ---- attention main loop (energy for ic+1 interleaved with AV of ic) ----
    postproc = []  # deferred (ic, isub, o_ps, obs) postprocessing closures

    def emit_postproc(ic, isub, o_ps, obs):
        rec = work.tile([128, 1], F32, name="rec", tag="rec", bufs=4)
        nc.vector.reciprocal(rec[:], o_ps[:, C:C + 1])
        osc = work.tile([128, C], F32, name="osc", tag="osc", bufs=3)
        nc.vector.tensor_scalar(
            osc[:], o_ps[:, 0:C], rec[:], gam[:],
            op0=mybir.AluOpType.mult, op1=mybir.AluOpType.mult)
        for cc in range(2):
            tp_ps = pspool.tile([128, 128], F32, name="tp_ps", tag="ops", bufs=2)
            nc.tensor.transpose(tp_ps[:], osc[:, bass.ts(cc, 128)], ident[:])
            nc.vector.tensor_add(
                obs[cc][:, bass.ts(isub, 128)], tp_ps[:],
                xf[cc][:, ic * IC + isub * 128: ic * IC + (isub + 1) * 128])

    for ic in range(NIC):
        isl = bass.ts(ic, IC)
        pt_next = []
        obs = [work.tile([128, IC], F32, name=f"ob{cc}_{ic}", tag=f"ob{cc}", bufs=2)
               for cc in range(2)]
        prev = None
        for isub in range(4):
            o_ps = pspool.tile([128, C + 1], F32, name="o_ps", tag="ops", bufs=2)
            for jb in range(NJB):
                nc.tensor.matmul(o_ps[:], pt_slice(pt_cur, jb, isub), vt[jb][:],
                                 start=(jb == 0), stop=(jb == NJB - 1))
            if ic + 1 < NIC:
                emit_energy_round(ic + 1, 2 * isub, pt_next)
                emit_energy_round(ic + 1, 2 * isub + 1, pt_next)
            if prev is not None:
                emit_postproc(ic, prev[0], prev[1], obs)
            prev = (isub, o_ps)
        emit_postproc(ic, prev[0], prev[1], obs)
        for cc in range(2):
            nc.sync.dma_start(out_d[cc * 128:(cc + 1) * 128, isl], obs[cc][:])
        pt_cur = pt_next

    # ---- label broadcast stores (ACT work deferred to the tail) ----
    for t in range(NIC):
        u_bc = work.tile([LC, IC], F32, name="u_bc", tag="u_bc", bufs=2)
        nc.scalar.activation(u_bc[:], xf[0][0:LC, bass.ts(t, IC)],
                             AF.Identity, bias=u_sb[:], scale=0.0)
        nc.sync.dma_start(out_d[C:COUT, bass.ts(t, IC)], u_bc[:])

    ctx.close()


_COMPILED = None


def _get_compiled():
    global _COMPILED
    if _COMPILED is None:
        nc = bacc.Bacc("TRN2", target_bir_lowering=False, debug=False)
        with tile.TileContext(nc) as tc:
            build_program(nc, tc)
        nc.compile()
        _COMPILED = nc
    return _COMPILED


def kernel(x, label, Wq, bq, Wk, bk, Wv, bv, gamma, We, be, _trace=False):
    x = np.asarray(x, np.float32)
    label = np.asarray(label, np.float32)
    Wq, bq = np.asarray(Wq, np.float32), np.asarray(bq, np.float32)
    Wk, bk = np.asarray(Wk, np.float32), np.asarray(bk, np.float32)
    Wv, bv = np.asarray(Wv, np.float32), np.asarray(bv, np.float32)
    gamma = np.asarray(gamma, np.float32)
    We, be = np.asarray(We, np.float32), np.asarray(be, np.float32)

    nc = _get_compiled()
    in_maps = [host_prep(x[b], label[b], Wq, bq, Wk, bk, Wv, bv, gamma, We, be)
               for b in range(B)]
    res = run_bass_kernel_spmd(nc, in_maps, list(range(B)), trace=_trace)
    out = np.stack([res.results[b]["out"] for b in range(B)])
    out = out.reshape(B, COUT, HW, HW).astype(np.float32)
    if _trace:
        return out, res
    return out
